# revision 38
# baseline (speedup 1.0000x reference)
"""Multi-head causal self-attention (B=4, S=2048, D=1024, H=16) on 8 NeuronCores.

Sharding: core c handles batch b=c//2 and heads [8*(c%2), 8*(c%2)+8) (tensor
parallel over heads x data parallel over batch). Each core computes its 8
heads' Q/K/V projections, causal attention, and a partial O-projection
(contracting only its 512 ctx dims). Host sums the two partial outputs per
batch.

Kernel math (per core):
  Q/K projections run as fp8e4m3 DoubleRow matmuls (x and wq*32 quantized to
  fp8 host-side; 2x PE rate via 256-deep contraction), evacuated to bf16.
  The 1/32^2 de-scale is folded into the exp activation scale.
  V projection runs in bf16 (x and wv bf16 host-side); V stored per k-tile
  with an appended ones-column so softmax denominators fall out of the PV
  matmul as one extra output row.
  scores (transposed): ST2[k, head, q] = KT_j.T @ QT per head into one
  2-bank PSUM tile, bf16 operands, exact causal widths (bf16 matmuls run
  1 cyc/row at any width). ONE exp per j covers both heads (halves ScalarE
  instruction-init overhead); 128x128 bf16 tri-mask multiply on DVE for
  diagonal tiles.
  PV: ctxT[65, q] += V_aug_j.T @ PT_j in bf16 (row 64 = denom).
  normalize off the critical path: evacuate ctx+denom rows to SBUF,
  reciprocal_approx_fast, gpsimd partition-broadcast, one DVE multiply.
  O-projection in f32r: out[s, D] = sum_ct ctxT_ct.T @ woT_ct, interleaved
  into pair 3's attention (per finished chunk) on the idle "pp" PSUM banks;
  full [128, D] rows -> contiguous 512KB DMAs round-robined over 3 queues.
"""
import sys
for _p in ('/opt/trn_rl_repo', '/root/.axon_site/_ro/trn_rl_repo'):
    if _p not in sys.path:
        sys.path.insert(0, _p)

import numpy as np

B, S, D, H = 4, 2048, 1024, 16
DH = 64
N_CORES = 8
HL = H // 2           # local heads per core
DL = HL * DH          # local ctx dims per core
SW = 32.0             # fp8 weight pre-scale for Q/K projections
QK_FP8 = True         # False: bf16 Q/K projections (safe fallback)


def build_nc(s=S, d=D, hl=HL, n_cores=N_CORES, reps=1):
    """Build the per-core Bass program (shapes parameterizable for sim tests)."""
    import concourse.bacc as bacc
    import concourse.mybir as mybir
    import concourse.tile as tile

    DT = mybir.dt
    F32 = DT.float32
    F32R = DT.float32r
    BF16 = DT.bfloat16
    F8 = DT.float8e4
    AFT = mybir.ActivationFunctionType
    PM = mybir.MatmulPerfMode

    dl = hl * DH
    n_kt = s // 128       # k/s tiles
    n_ch = s // 512       # 512-wide q chunks
    n_dt = d // 128       # d_model tiles
    n_oc = d // 512       # output d chunks
    pairs = hl // 2
    exp_scale = 0.125 / (SW * SW) if QK_FP8 else 0.125
    WDT = F8 if QK_FP8 else BF16

    nc = bacc.Bacc("TRN2", target_bir_lowering=False, debug=False,
                   num_devices=n_cores)
    # host-prepacked layouts (contiguous DMA blocks)
    xb = nc.declare_dram_parameter("xb", [n_dt, 128, s], BF16, isOutput=False)
    if QK_FP8:
        x8 = nc.declare_dram_parameter("x8", [n_dt, 128, s], F8, isOutput=False)
    wqP = nc.declare_dram_parameter("wqP", [128, pairs, n_dt, 128], WDT,
                                    isOutput=False)
    wkP = nc.declare_dram_parameter("wkP", [128, pairs, n_dt, 128], WDT,
                                    isOutput=False)
    wvP = nc.declare_dram_parameter("wvP", [n_dt, 128, dl], BF16, isOutput=False)
    woT = nc.declare_dram_parameter("woT", [dl, d], F32R, isOutput=False)
    tri = nc.declare_dram_parameter("tri", [128, 128], BF16, isOutput=False)
    out = nc.declare_dram_parameter("out", [s, d], F32, isOutput=True)

    with tile.TileContext(nc) as tc:
        with tc.tile_pool(name="persist", bufs=1) as pp, \
             tc.tile_pool(name="stream", bufs=1) as sp, \
             tc.tile_pool(name="psum", bufs=1, space="PSUM") as ps:

            # ---- resident tensors ----
            # v_sb: DH+2 bf16 cols per head so the ones column sits 4B-aligned
            # (cols DH:DH+2 memset as one packed f32; PV reads 0:DH+1)
            xb_sb = pp.tile([128, n_dt, s], BF16, name="xb_sb")
            if QK_FP8:
                x8_sb = pp.tile([128, n_dt, s], F8, name="x8_sb")
            else:
                x8_sb = xb_sb
            v_sb = pp.tile([128, n_kt, hl, DH + 2], BF16, name="v_sb")
            ctx_all = pp.tile([128, pairs, s], F32R, name="ctx_all")
            qk_all = pp.tile([128, 2, pairs, s], BF16, name="qk_all")
            tri2 = pp.tile([128, 2, 128], BF16, name="tri2")

            nc.gpsimd.dma_start(out=tri2[:, 0, :], in_=tri[:, :])
            nc.gpsimd.dma_start(out=tri2[:, 1, :], in_=tri[:, :])

            for _rep in range(reps):
              R = f"{_rep}_" if reps > 1 else ""
              wq_sb = sp.tile([128, pairs, n_dt, 128], WDT, name=f"{R}wq_sb",
                              tag="wq")
              wk_sb = sp.tile([128, pairs, n_dt, 128], WDT, name=f"{R}wk_sb",
                              tag="wk")
              wv_sb = sp.tile([128, n_dt, dl], BF16, name=f"{R}wv_sb", tag="wv")
              wo_sb = sp.tile([128, pairs, d], F32R, name=f"{R}wo_sb", tag="wo")

              # ---- upfront DMA ----
              # The Activation queue must stay clear for the exp stream (each
              # DMA issue ties its sequencer ~630ns), but the first exp isn't
              # needed until ~15us in: let scalar take a handful of the
              # init-critical issues, then alternate sync (HWDGE) / gpsimd
              # (SWDGE).
              QQ = (nc.sync, nc.gpsimd)
              qi = [0]

              # preload the Exp activation table before scalar's DMA issues
              warm = sp.tile([1, 2], F32, name=f"{R}warm", tag="warm", bufs=1)
              nc.scalar.activation(out=warm, in_=tri2[0:1, 0, 0:2],
                                   func=AFT.Exp, scale=1.0)

              def dma(dst, src):
                  if qi[0] % 3 == 1 and qi[0] < 18:
                      nc.scalar.dma_start(out=dst, in_=src)
                  else:
                      QQ[(qi[0] % 3) > 0].dma_start(out=dst, in_=src)
                  qi[0] += 1

              # v_sb ones-columns first (Pool op; V evacs wait on it)
              # two bf16 ones packed as one f32 (bits 0x3F803F80)
              ones2 = float(np.frombuffer(np.uint32(0x3F803F80).tobytes(),
                                          dtype=np.float32)[0])
              nc.gpsimd.memset(v_sb[:, :, :, DH:DH + 2].bitcast(F32), ones2)

              # proj-chunk-0 gate first at fine granularity (pair-0 weights +
              # first x8 strips land within ~2.5us), then V-group-0 gate,
              # then the rest coarsened to one transfer per (tile, stream)
              for p_ in range(pairs):
                  dma(wq_sb[:, p_, :, :], wqP[:, p_, :, :])
                  if QK_FP8 and p_ * 2 < n_dt:
                      dma(x8_sb[:, 2 * p_, 0:512], x8[2 * p_, :, 0:512])
                      dma(x8_sb[:, 2 * p_ + 1, 0:512], x8[2 * p_ + 1, :, 0:512])
              for p_ in range(pairs):
                  dma(wk_sb[:, p_, :, :], wkP[:, p_, :, :])
              if QK_FP8:
                  for t in range(2 * pairs, n_dt):
                      dma(x8_sb[:, t, 0:512], x8[t, :, 0:512])
              for t in range(n_dt):
                  dma(wv_sb[:, t, :], wvP[t])
                  dma(xb_sb[:, t, 0:512], xb[t, :, 0:512])
              if n_ch > 1:
                  for t in range(n_dt):
                      dma(xb_sb[:, t, 512:s], xb[t, :, 512:s])
                      if QK_FP8:
                          dma(x8_sb[:, t, 512:s], x8[t, :, 512:s])
              dma(wo_sb,
                  woT.rearrange("(c r) d -> r c d", r=128))

              # ---- building blocks ----
              def proj(g, p, c4):
                  """Q (g=0) / K (g=1) projection of pair p, chunk c4."""
                  w_sb = wq_sb if g == 0 else wk_sb
                  psx = ps.tile([128, 512], F32, name=f"{R}ps{g}_{p}_{c4}",
                                tag="pp", bufs=2)
                  cs = slice(512 * c4, 512 * (c4 + 1))
                  if QK_FP8:
                      for t2 in range(n_dt // 2):
                          nc.tensor.matmul(psx[:, :],
                                           w_sb[:, p, 2 * t2:2 * t2 + 2, :],
                                           x8_sb[:, 2 * t2:2 * t2 + 2, cs],
                                           start=(t2 == 0),
                                           stop=(t2 == n_dt // 2 - 1),
                                           perf_mode=PM.DoubleRow)
                  else:
                      for t in range(n_dt):
                          nc.tensor.matmul(psx[:, :], w_sb[:, p, t, :],
                                           xb_sb[:, t, cs],
                                           start=(t == 0), stop=(t == n_dt - 1))
                  nc.vector.tensor_copy(out=qk_all[:, g, p, cs], in_=psx)

              def v_proj(kt):
                  pv = ps.tile([128, dl], F32, name=f"{R}pv_{kt}",
                               tag="pp", bufs=2)
                  for t in range(n_dt):
                      nc.tensor.matmul(pv[:, :],
                                       xb_sb[:, t, 128 * kt:128 * (kt + 1)],
                                       wv_sb[:, t, :],
                                       start=(t == 0), stop=(t == n_dt - 1))
                  nc.vector.tensor_copy(
                      out=v_sb[:, kt, :, 0:DH],
                      in_=pv.rearrange("p (h e) -> p h e", e=DH))

              def attn(p, c4):
                  q0 = 512 * c4
                  qt = qk_all[:, 0, p, :]
                  kt_sb = qk_all[:, 1, p, :]
                  ctxA = ps.tile([DH + 1, 512], F32, name=f"{R}cA_{p}_{c4}",
                                 tag="ctxA", bufs=1)
                  ctxB = ps.tile([DH + 1, 512], F32, name=f"{R}cB_{p}_{c4}",
                                 tag="ctxB", bufs=1)
                  nj = 4 * c4 + 4
                  pending = None
                  for j in range(nj):
                      m = j - 4 * c4
                      n0 = 128 * m if m >= 0 else 0
                      st2 = ps.tile([128, 2, 512], F32, name=f"{R}s_{p}_{c4}_{j}",
                                    tag="st", bufs=2)
                      ks = slice(128 * j, 128 * (j + 1))
                      qs = slice(q0 + n0, q0 + 512)
                      nc.tensor.matmul(st2[:, 0, n0:512], kt_sb[0:64, ks],
                                       qt[0:64, qs], start=True, stop=True)
                      nc.tensor.matmul(st2[:, 1, n0:512], kt_sb[64:128, ks],
                                       qt[64:128, qs], start=True, stop=True)
                      pt2 = sp.tile([128, 2, 512], BF16, name=f"{R}p_{p}_{c4}_{j}",
                                    tag="pt", bufs=3)
                      nc.scalar.activation(out=pt2[:, :, n0:512],
                                           in_=st2[:, :, n0:512],
                                           func=AFT.Exp, scale=exp_scale)
                      if m >= 0:
                          nc.vector.tensor_mul(pt2[:, :, n0:n0 + 128],
                                               pt2[:, :, n0:n0 + 128], tri2)
                      if pending is not None:
                          _emit_pv(nc, v_sb, ctxA, ctxB, p, pending, nj)
                      pending = (j, pt2, n0)
                  _emit_pv(nc, v_sb, ctxA, ctxB, p, pending, nj)

                  # evacuate ctx rows to SBUF scratch (frees the ctx bank
                  # after two short copies); normalize off the critical path:
                  # fast reciprocal of the denom row, gpsimd partition-
                  # broadcast, one multiply into ctx_all. Partition offsets
                  # per op follow baseline-proven patterns (out==ins offset
                  # for TensorTensor; cross-offset only on TensorCopy).
                  for head, cpsum in ((0, ctxA), (1, ctxB)):
                      r_i = 2 * c4 + head
                      hs = slice(64 * head, 64 * head + 64)
                      cn = sp.tile([128, 512], F32, name=f"{R}cn_{p}_{r_i}",
                                   tag="cn", bufs=2)
                      nc.vector.tensor_copy(out=cn[hs, :], in_=cpsum[0:DH, :])
                      dn1 = sp.tile([1, 512], F32, name=f"{R}dn_{p}_{r_i}",
                                    tag="dn", bufs=2)
                      nc.scalar.copy(out=dn1, in_=cpsum[DH:DH + 1, :])
                      rb1 = sp.tile([1, 512], F32, name=f"{R}rc_{p}_{r_i}",
                                    tag="rc", bufs=1)
                      nc.vector.reciprocal_approx_fast(out=rb1, in_=dn1)
                      rb = sp.tile([128, 512], F32, name=f"{R}rb_{p}_{r_i}",
                                   tag="rb", bufs=1)
                      nc.gpsimd.partition_broadcast(rb, rb1)
                      nc.vector.tensor_mul(
                          ctx_all[hs, p, q0:q0 + 512],
                          cn[hs, :], rb[hs, :])

              def o_proj(st_i):
                  ot = sp.tile([128, d], F32, name=f"{R}ot_{st_i}",
                               tag="ot", bufs=3)
                  for oc in range(n_oc):
                      pso = ps.tile([128, 512], F32, name=f"{R}po_{st_i}_{oc}",
                                    tag="pp", bufs=2)
                      for ct in range(pairs):
                          nc.tensor.matmul(
                              pso[:, :],
                              ctx_all[:, ct, 128 * st_i:128 * (st_i + 1)],
                              wo_sb[:, ct, 512 * oc:512 * (oc + 1)],
                              start=(ct == 0), stop=(ct == pairs - 1))
                      nc.vector.tensor_copy(
                          out=ot[:, 512 * oc:512 * (oc + 1)], in_=pso)
                  QQ[st_i % 2].dma_start(
                      out=out[128 * st_i:128 * (st_i + 1), :], in_=ot)

              # ---- chunk-major schedule ----
              # init: chunk-0 projections (smallest DMA gate), then V group 0
              for p in range(pairs):
                  proj(0, p, 0)
                  proj(1, p, 0)
              for kt in range(4):
                  v_proj(kt)

              # Attention stretches are Act-paced (exp: 1.67W + ~190ns per
              # j-step vs PE's 1.67W): PE idles at each pair's tail waiting
              # for the exp queue. Fill those bubbles with matmul-only work
              # (next chunk's projections + V group, previous chunk's O
              # tiles) spread BETWEEN the pair attentions of each stretch.
              for c4 in range(n_ch):
                  fills = []
                  if c4 > 0:
                      fills += [lambda st=st: o_proj(st)
                                for st in range(4 * (c4 - 1), 4 * c4)]
                  if c4 < n_ch - 1:
                      fills += [lambda g=g, p=p, c=c4 + 1: proj(g, p, c)
                                for p in range(pairs) for g in (0, 1)]
                      fills += [lambda kt=kt: v_proj(kt)
                                for kt in range(4 * (c4 + 1), 4 * (c4 + 2))]
                  # Act's per-unit deficit grows with the pair index inside a
                  # stretch; on the last chunk skew the fills toward the end
                  # so PE still has work under the largest exp backlog
                  if c4 == n_ch - 1:
                      w = [0, 0, 1, 2, 4][:pairs + 1]
                  else:
                      w = list(range(pairs + 1))
                  tot = w[-1] if w[-1] else 1
                  for p in range(pairs):
                      attn(p, c4)
                      lo = len(fills) * w[p] // tot
                      hi = len(fills) * w[p + 1] // tot
                      for f in fills[lo:hi]:
                          f()
                  # last chunk: final O tiles come after the whole stretch
                  if c4 == n_ch - 1:
                      for st in range(4 * c4, 4 * c4 + 4):
                          o_proj(st)

    nc.compile()
    return nc


def _emit_pv(nc, v_sb, ctxA, ctxB, p, pending, nj):
    j, pt2, n0 = pending
    start = (j == 0)
    stop = (j == nj - 1)
    nc.tensor.matmul(ctxA[:, n0:512], v_sb[:, j, 2 * p, 0:DH + 1],
                     pt2[:, 0, n0:512], start=start, stop=stop)
    nc.tensor.matmul(ctxB[:, n0:512], v_sb[:, j, 2 * p + 1, 0:DH + 1],
                     pt2[:, 1, n0:512], start=start, stop=stop)


def make_tri():
    import ml_dtypes
    k = np.arange(128)[:, None]
    q = np.arange(128)[None, :]
    return (k <= q).astype(ml_dtypes.bfloat16)


def shard_inputs(in_features, q_weight, k_weight, v_weight, o_weight):
    """-> list of 8 per-core input dicts (host-side repack + quantize)."""
    import ml_dtypes
    BF = ml_dtypes.bfloat16
    F8 = ml_dtypes.float8_e4m3
    n_dt = D // 128
    pairs = HL // 2
    tri = make_tri()
    maps = []
    for c in range(N_CORES):
        b, g = divmod(c, 2)
        hs = slice(DL * g, DL * (g + 1))   # local head dims in the full D
        xT = in_features[b].T              # [D, S]
        xTt = np.ascontiguousarray(xT.reshape(n_dt, 128, S))
        wq = q_weight[hs, :]               # [DL, D]
        wk = k_weight[hs, :]
        # wP[r, p, t, m] = w[128p+m, 128t+r]
        wqP = np.ascontiguousarray(
            wq.reshape(pairs, 128, n_dt, 128).transpose(3, 0, 2, 1))
        wkP = np.ascontiguousarray(
            wk.reshape(pairs, 128, n_dt, 128).transpose(3, 0, 2, 1))
        m = {
            "xb": xTt.astype(BF),
            "wvP": np.ascontiguousarray(
                v_weight[hs, :].T.reshape(n_dt, 128, DL)).astype(BF),
            "woT": np.ascontiguousarray(o_weight[:, hs].T),
            "tri": tri,
        }
        if QK_FP8:
            m["x8"] = xTt.astype(F8)
            m["wqP"] = (wqP * SW).astype(F8)
            m["wkP"] = (wkP * SW).astype(F8)
        else:
            m["wqP"] = wqP.astype(BF)
            m["wkP"] = wkP.astype(BF)
        maps.append(m)
    return maps


def gather_output(results):
    """results: list of 8 dicts with 'out' [S, D] partials -> [B, S, D]."""
    return np.stack([results[2 * b]["out"] + results[2 * b + 1]["out"]
                     for b in range(B)])


_nc_cache = {}


def kernel(in_features, q_weight, k_weight, v_weight, o_weight):
    from concourse.bass_utils import run_bass_kernel_spmd
    if "nc" not in _nc_cache:
        _nc_cache["nc"] = build_nc()
    nc = _nc_cache["nc"]
    in_maps = shard_inputs(np.asarray(in_features, dtype=np.float32),
                           np.asarray(q_weight, dtype=np.float32),
                           np.asarray(k_weight, dtype=np.float32),
                           np.asarray(v_weight, dtype=np.float32),
                           np.asarray(o_weight, dtype=np.float32))
    res = run_bass_kernel_spmd(nc, in_maps, core_ids=list(range(N_CORES)))
    return gather_output(res.results)


# revision 41
# speedup vs baseline: 1.0498x; 1.0498x over previous
"""Multi-head causal self-attention (B=4, S=2048, D=1024, H=16) on 8 NeuronCores.

Sharding: core c handles batch b=c//2 and heads [8*(c%2), 8*(c%2)+8) (tensor
parallel over heads x data parallel over batch). Each core computes its 8
heads' Q/K/V projections, causal attention, and a partial O-projection
(contracting only its 512 ctx dims). Host sums the two partial outputs per
batch.

Kernel math (per core):
  Q/K projections run as fp8e4m3 DoubleRow matmuls (x and wq*32 quantized to
  fp8 host-side; 2x PE rate via 256-deep contraction), evacuated to bf16.
  The 1/32^2 de-scale is folded into the exp activation scale.
  V projection runs in bf16 (x and wv bf16 host-side); V stored per k-tile
  with an appended ones-column so softmax denominators fall out of the PV
  matmul as one extra output row.
  scores (transposed): ST2[k, head, q] = KT_j.T @ QT per head into one
  2-bank PSUM tile, bf16 operands, exact causal widths (bf16 matmuls run
  1 cyc/row at any width). ONE exp per j covers both heads (halves ScalarE
  instruction-init overhead); 128x128 bf16 tri-mask multiply on DVE for
  diagonal tiles.
  PV: ctxT[65, q] += V_aug_j.T @ PT_j in bf16 (row 64 = denom).
  normalize off the critical path: evacuate ctx+denom rows to SBUF,
  reciprocal_approx_fast, gpsimd partition-broadcast, one DVE multiply.
  O-projection in f32r: out[s, D] = sum_ct ctxT_ct.T @ woT_ct, interleaved
  into pair 3's attention (per finished chunk) on the idle "pp" PSUM banks;
  full [128, D] rows -> contiguous 512KB DMAs round-robined over 3 queues.
"""
import sys
for _p in ('/opt/trn_rl_repo', '/root/.axon_site/_ro/trn_rl_repo'):
    if _p not in sys.path:
        sys.path.insert(0, _p)

import numpy as np

B, S, D, H = 4, 2048, 1024, 16
DH = 64
N_CORES = 8
HL = H // 2           # local heads per core
DL = HL * DH          # local ctx dims per core
SW = 32.0             # fp8 weight pre-scale for Q/K projections
QK_FP8 = True         # False: bf16 Q/K projections (safe fallback)


def build_nc(s=S, d=D, hl=HL, n_cores=N_CORES, reps=1):
    """Build the per-core Bass program (shapes parameterizable for sim tests)."""
    import concourse.bacc as bacc
    import concourse.mybir as mybir
    import concourse.tile as tile

    DT = mybir.dt
    F32 = DT.float32
    F32R = DT.float32r
    BF16 = DT.bfloat16
    F8 = DT.float8e4
    AFT = mybir.ActivationFunctionType
    PM = mybir.MatmulPerfMode

    dl = hl * DH
    n_kt = s // 128       # k/s tiles
    n_ch = s // 512       # 512-wide q chunks
    n_dt = d // 128       # d_model tiles
    n_oc = d // 512       # output d chunks
    pairs = hl // 2
    exp_scale = 0.125 / (SW * SW) if QK_FP8 else 0.125
    WDT = F8 if QK_FP8 else BF16

    nc = bacc.Bacc("TRN2", target_bir_lowering=False, debug=False,
                   num_devices=n_cores)
    # host-prepacked layouts (contiguous DMA blocks)
    xb = nc.declare_dram_parameter("xb", [n_dt, 128, s], BF16, isOutput=False)
    if QK_FP8:
        x8 = nc.declare_dram_parameter("x8", [n_dt, 128, s], F8, isOutput=False)
    wqP = nc.declare_dram_parameter("wqP", [128, pairs, n_dt, 128], WDT,
                                    isOutput=False)
    wkP = nc.declare_dram_parameter("wkP", [128, pairs, n_dt, 128], WDT,
                                    isOutput=False)
    wvP = nc.declare_dram_parameter("wvP", [n_dt, 128, dl], BF16, isOutput=False)
    woT = nc.declare_dram_parameter("woT", [dl, d], F32R, isOutput=False)
    tri = nc.declare_dram_parameter("tri", [128, 128], BF16, isOutput=False)
    out = nc.declare_dram_parameter("out", [s, d], F32, isOutput=True)

    with tile.TileContext(nc) as tc:
        with tc.tile_pool(name="persist", bufs=1) as pp, \
             tc.tile_pool(name="stream", bufs=1) as sp, \
             tc.tile_pool(name="psum", bufs=1, space="PSUM") as ps:

            # ---- resident tensors ----
            # v_sb: DH+2 bf16 cols per head so the ones column sits 4B-aligned
            # (cols DH:DH+2 memset as one packed f32; PV reads 0:DH+1)
            xb_sb = pp.tile([128, n_dt, s], BF16, name="xb_sb")
            if QK_FP8:
                x8_sb = pp.tile([128, n_dt, s], F8, name="x8_sb")
            else:
                x8_sb = xb_sb
            ctx_all = pp.tile([128, pairs, s], F32R, name="ctx_all")
            qk_all = pp.tile([128, 2, pairs, s], BF16, name="qk_all")
            tri2 = pp.tile([128, 2, 128], BF16, name="tri2")

            nc.gpsimd.dma_start(out=tri2[:, 0, :], in_=tri[:, :])
            nc.gpsimd.dma_start(out=tri2[:, 1, :], in_=tri[:, :])

            for _rep in range(reps):
              R = f"{_rep}_" if reps > 1 else ""
              wq_sb = sp.tile([128, pairs, n_dt, 128], WDT, name=f"{R}wq_sb",
                              tag="wq")
              wk_sb = sp.tile([128, pairs, n_dt, 128], WDT, name=f"{R}wk_sb",
                              tag="wk")
              wv_sb = sp.tile([128, n_dt, dl], BF16, name=f"{R}wv_sb", tag="wv")
              wo_sb = sp.tile([128, pairs, d], F32R, name=f"{R}wo_sb", tag="wo")
              # double-buffered so rep r+1's V projection does not serialize
              # on rep r's final attention chunk (which reads all V tiles)
              v_sb = sp.tile([128, n_kt, hl, DH + 2], BF16, name=f"{R}v_sb",
                             tag="v", bufs=2)

              # ---- upfront DMA ----
              # The Activation queue must stay clear for the exp stream (each
              # DMA issue ties its sequencer ~630ns), but the first exp isn't
              # needed until ~15us in: let scalar take a handful of the
              # init-critical issues, then alternate sync (HWDGE) / gpsimd
              # (SWDGE).
              QQ = (nc.sync, nc.gpsimd)
              qi = [0]

              # preload the Exp activation table before scalar's DMA issues
              warm = sp.tile([1, 2], F32, name=f"{R}warm", tag="warm", bufs=1)
              nc.scalar.activation(out=warm, in_=tri2[0:1, 0, 0:2],
                                   func=AFT.Exp, scale=1.0)

              def dma(dst, src):
                  if qi[0] % 3 == 1 and qi[0] < 18:
                      nc.scalar.dma_start(out=dst, in_=src)
                  else:
                      QQ[(qi[0] % 3) > 0].dma_start(out=dst, in_=src)
                  qi[0] += 1

              # v_sb ones-columns first (Pool op; V evacs wait on it)
              # two bf16 ones packed as one f32 (bits 0x3F803F80)
              ones2 = float(np.frombuffer(np.uint32(0x3F803F80).tobytes(),
                                          dtype=np.float32)[0])
              nc.gpsimd.memset(v_sb[:, :, :, DH:DH + 2].bitcast(F32), ones2)

              # proj-chunk-0 gate first at fine granularity (pair-0 weights +
              # first x8 strips land within ~2.5us), then V-group-0 gate,
              # then the rest coarsened to one transfer per (tile, stream)
              for p_ in range(pairs):
                  dma(wq_sb[:, p_, :, :], wqP[:, p_, :, :])
                  if QK_FP8 and p_ * 2 < n_dt:
                      dma(x8_sb[:, 2 * p_, 0:512], x8[2 * p_, :, 0:512])
                      dma(x8_sb[:, 2 * p_ + 1, 0:512], x8[2 * p_ + 1, :, 0:512])
              for p_ in range(pairs):
                  dma(wk_sb[:, p_, :, :], wkP[:, p_, :, :])
              if QK_FP8:
                  for t in range(2 * pairs, n_dt):
                      dma(x8_sb[:, t, 0:512], x8[t, :, 0:512])
              for t in range(n_dt):
                  dma(wv_sb[:, t, :], wvP[t])
                  dma(xb_sb[:, t, 0:512], xb[t, :, 0:512])
              if n_ch > 1:
                  for t in range(n_dt):
                      dma(xb_sb[:, t, 512:s], xb[t, :, 512:s])
                      if QK_FP8:
                          dma(x8_sb[:, t, 512:s], x8[t, :, 512:s])
              dma(wo_sb,
                  woT.rearrange("(c r) d -> r c d", r=128))

              # ---- building blocks ----
              def proj(g, p, c4):
                  """Q (g=0) / K (g=1) projection of pair p, chunk c4."""
                  w_sb = wq_sb if g == 0 else wk_sb
                  psx = ps.tile([128, 512], F32, name=f"{R}ps{g}_{p}_{c4}",
                                tag="pp", bufs=2)
                  cs = slice(512 * c4, 512 * (c4 + 1))
                  if QK_FP8:
                      for t2 in range(n_dt // 2):
                          nc.tensor.matmul(psx[:, :],
                                           w_sb[:, p, 2 * t2:2 * t2 + 2, :],
                                           x8_sb[:, 2 * t2:2 * t2 + 2, cs],
                                           start=(t2 == 0),
                                           stop=(t2 == n_dt // 2 - 1),
                                           perf_mode=PM.DoubleRow)
                  else:
                      for t in range(n_dt):
                          nc.tensor.matmul(psx[:, :], w_sb[:, p, t, :],
                                           xb_sb[:, t, cs],
                                           start=(t == 0), stop=(t == n_dt - 1))
                  nc.vector.tensor_copy(out=qk_all[:, g, p, cs], in_=psx)

              def v_proj(kt):
                  pv = ps.tile([128, dl], F32, name=f"{R}pv_{kt}",
                               tag="pp", bufs=2)
                  for t in range(n_dt):
                      nc.tensor.matmul(pv[:, :],
                                       xb_sb[:, t, 128 * kt:128 * (kt + 1)],
                                       wv_sb[:, t, :],
                                       start=(t == 0), stop=(t == n_dt - 1))
                  nc.vector.tensor_copy(
                      out=v_sb[:, kt, :, 0:DH],
                      in_=pv.rearrange("p (h e) -> p h e", e=DH))

              def attn(p, c4):
                  q0 = 512 * c4
                  qt = qk_all[:, 0, p, :]
                  kt_sb = qk_all[:, 1, p, :]
                  ctxA = ps.tile([DH + 1, 512], F32, name=f"{R}cA_{p}_{c4}",
                                 tag="ctxA", bufs=1)
                  ctxB = ps.tile([DH + 1, 512], F32, name=f"{R}cB_{p}_{c4}",
                                 tag="ctxB", bufs=1)
                  nj = 4 * c4 + 4
                  pending = None
                  for j in range(nj):
                      m = j - 4 * c4
                      n0 = 128 * m if m >= 0 else 0
                      st2 = ps.tile([128, 2, 512], F32, name=f"{R}s_{p}_{c4}_{j}",
                                    tag="st", bufs=2)
                      ks = slice(128 * j, 128 * (j + 1))
                      qs = slice(q0 + n0, q0 + 512)
                      nc.tensor.matmul(st2[:, 0, n0:512], kt_sb[0:64, ks],
                                       qt[0:64, qs], start=True, stop=True)
                      nc.tensor.matmul(st2[:, 1, n0:512], kt_sb[64:128, ks],
                                       qt[64:128, qs], start=True, stop=True)
                      pt2 = sp.tile([128, 2, 512], BF16, name=f"{R}p_{p}_{c4}_{j}",
                                    tag="pt", bufs=3)
                      nc.scalar.activation(out=pt2[:, :, n0:512],
                                           in_=st2[:, :, n0:512],
                                           func=AFT.Exp, scale=exp_scale)
                      if m >= 0:
                          nc.vector.tensor_mul(pt2[:, :, n0:n0 + 128],
                                               pt2[:, :, n0:n0 + 128], tri2)
                      if pending is not None:
                          _emit_pv(nc, v_sb, ctxA, ctxB, p, pending, nj)
                      pending = (j, pt2, n0)
                  _emit_pv(nc, v_sb, ctxA, ctxB, p, pending, nj)

                  # evacuate ctx rows to SBUF scratch (frees the ctx bank
                  # after two short copies); normalize off the critical path:
                  # fast reciprocal of the denom row, gpsimd partition-
                  # broadcast, one multiply into ctx_all. Partition offsets
                  # per op follow baseline-proven patterns (out==ins offset
                  # for TensorTensor; cross-offset only on TensorCopy).
                  for head, cpsum in ((0, ctxA), (1, ctxB)):
                      r_i = 2 * c4 + head
                      hs = slice(64 * head, 64 * head + 64)
                      cn = sp.tile([128, 512], F32, name=f"{R}cn_{p}_{r_i}",
                                   tag="cn", bufs=2)
                      nc.vector.tensor_copy(out=cn[hs, :], in_=cpsum[0:DH, :])
                      dn1 = sp.tile([1, 512], F32, name=f"{R}dn_{p}_{r_i}",
                                    tag="dn", bufs=2)
                      nc.scalar.copy(out=dn1, in_=cpsum[DH:DH + 1, :])
                      rb1 = sp.tile([1, 512], F32, name=f"{R}rc_{p}_{r_i}",
                                    tag="rc", bufs=1)
                      nc.vector.reciprocal_approx_fast(out=rb1, in_=dn1)
                      rb = sp.tile([128, 512], F32, name=f"{R}rb_{p}_{r_i}",
                                   tag="rb", bufs=1)
                      nc.gpsimd.partition_broadcast(rb, rb1)
                      nc.vector.tensor_mul(
                          ctx_all[hs, p, q0:q0 + 512],
                          cn[hs, :], rb[hs, :])

              def o_proj(st_i):
                  ot = sp.tile([128, d], F32, name=f"{R}ot_{st_i}",
                               tag="ot", bufs=3)
                  for oc in range(n_oc):
                      pso = ps.tile([128, 512], F32, name=f"{R}po_{st_i}_{oc}",
                                    tag="pp", bufs=2)
                      for ct in range(pairs):
                          nc.tensor.matmul(
                              pso[:, :],
                              ctx_all[:, ct, 128 * st_i:128 * (st_i + 1)],
                              wo_sb[:, ct, 512 * oc:512 * (oc + 1)],
                              start=(ct == 0), stop=(ct == pairs - 1))
                      nc.vector.tensor_copy(
                          out=ot[:, 512 * oc:512 * (oc + 1)], in_=pso)
                  QQ[st_i % 2].dma_start(
                      out=out[128 * st_i:128 * (st_i + 1), :], in_=ot)

              # ---- chunk-major schedule ----
              # init: chunk-0 projections (smallest DMA gate), then V group 0
              for p in range(pairs):
                  proj(0, p, 0)
                  proj(1, p, 0)
              for kt in range(4):
                  v_proj(kt)

              # Attention stretches are Act-paced (exp: 1.67W + ~190ns per
              # j-step vs PE's 1.67W): PE idles at each pair's tail waiting
              # for the exp queue. Fill those bubbles with matmul-only work
              # (next chunk's projections + V group, previous chunk's O
              # tiles) spread BETWEEN the pair attentions of each stretch.
              # Stretches 0..n_ch-2 are PE-bound with just the next chunk's
              # projections + V group as fill; ALL O-projection fill goes to
              # the last stretch, where the Act (exp) backlog peaks.
              for c4 in range(n_ch):
                  fills = []
                  if c4 < n_ch - 1:
                      fills += [lambda g=g, p=p, c=c4 + 1: proj(g, p, c)
                                for p in range(pairs) for g in (0, 1)]
                      fills += [lambda kt=kt: v_proj(kt)
                                for kt in range(4 * (c4 + 1), 4 * (c4 + 2))]
                  else:
                      fills += [lambda st=st: o_proj(st)
                                for st in range(0, 4 * c4)]
                  for p in range(pairs):
                      attn(p, c4)
                      lo = len(fills) * p // pairs
                      hi = len(fills) * (p + 1) // pairs
                      for f in fills[lo:hi]:
                          f()
                  # last chunk: final O tiles come after the whole stretch
                  if c4 == n_ch - 1:
                      for st in range(4 * c4, 4 * c4 + 4):
                          o_proj(st)

    nc.compile()
    return nc


def _emit_pv(nc, v_sb, ctxA, ctxB, p, pending, nj):
    j, pt2, n0 = pending
    start = (j == 0)
    stop = (j == nj - 1)
    nc.tensor.matmul(ctxA[:, n0:512], v_sb[:, j, 2 * p, 0:DH + 1],
                     pt2[:, 0, n0:512], start=start, stop=stop)
    nc.tensor.matmul(ctxB[:, n0:512], v_sb[:, j, 2 * p + 1, 0:DH + 1],
                     pt2[:, 1, n0:512], start=start, stop=stop)


def make_tri():
    import ml_dtypes
    k = np.arange(128)[:, None]
    q = np.arange(128)[None, :]
    return (k <= q).astype(ml_dtypes.bfloat16)


def shard_inputs(in_features, q_weight, k_weight, v_weight, o_weight):
    """-> list of 8 per-core input dicts (host-side repack + quantize)."""
    import ml_dtypes
    BF = ml_dtypes.bfloat16
    F8 = ml_dtypes.float8_e4m3
    n_dt = D // 128
    pairs = HL // 2
    tri = make_tri()
    maps = []
    for c in range(N_CORES):
        b, g = divmod(c, 2)
        hs = slice(DL * g, DL * (g + 1))   # local head dims in the full D
        xT = in_features[b].T              # [D, S]
        xTt = np.ascontiguousarray(xT.reshape(n_dt, 128, S))
        wq = q_weight[hs, :]               # [DL, D]
        wk = k_weight[hs, :]
        # wP[r, p, t, m] = w[128p+m, 128t+r]
        wqP = np.ascontiguousarray(
            wq.reshape(pairs, 128, n_dt, 128).transpose(3, 0, 2, 1))
        wkP = np.ascontiguousarray(
            wk.reshape(pairs, 128, n_dt, 128).transpose(3, 0, 2, 1))
        m = {
            "xb": xTt.astype(BF),
            "wvP": np.ascontiguousarray(
                v_weight[hs, :].T.reshape(n_dt, 128, DL)).astype(BF),
            "woT": np.ascontiguousarray(o_weight[:, hs].T),
            "tri": tri,
        }
        if QK_FP8:
            m["x8"] = xTt.astype(F8)
            m["wqP"] = (wqP * SW).astype(F8)
            m["wkP"] = (wkP * SW).astype(F8)
        else:
            m["wqP"] = wqP.astype(BF)
            m["wkP"] = wkP.astype(BF)
        maps.append(m)
    return maps


def gather_output(results):
    """results: list of 8 dicts with 'out' [S, D] partials -> [B, S, D]."""
    return np.stack([results[2 * b]["out"] + results[2 * b + 1]["out"]
                     for b in range(B)])


_nc_cache = {}


def kernel(in_features, q_weight, k_weight, v_weight, o_weight):
    from concourse.bass_utils import run_bass_kernel_spmd
    if "nc" not in _nc_cache:
        _nc_cache["nc"] = build_nc()
    nc = _nc_cache["nc"]
    in_maps = shard_inputs(np.asarray(in_features, dtype=np.float32),
                           np.asarray(q_weight, dtype=np.float32),
                           np.asarray(k_weight, dtype=np.float32),
                           np.asarray(v_weight, dtype=np.float32),
                           np.asarray(o_weight, dtype=np.float32))
    res = run_bass_kernel_spmd(nc, in_maps, core_ids=list(range(N_CORES)))
    return gather_output(res.results)


# revision 46
# speedup vs baseline: 1.2544x; 1.1949x over previous
"""Multi-head causal self-attention (B=4, S=2048, D=1024, H=16) on 8 NeuronCores.

Sharding: core c handles batch b=c//2 and heads [8*(c%2), 8*(c%2)+8) (tensor
parallel over heads x data parallel over batch). Each core computes its 8
heads' Q/K/V projections, causal attention, and a partial O-projection
(contracting only its 512 ctx dims). Host sums the two partial outputs per
batch.

Kernel math (per core):
  Q/K projections run as fp8e4m3 DoubleRow matmuls (x and wq*32 quantized to
  fp8 host-side; 2x PE rate via 256-deep contraction), evacuated to bf16.
  The 1/32^2 de-scale is folded into the exp activation scale.
  V projection runs in bf16 (x and wv bf16 host-side); V stored per k-tile
  with an appended ones-column so softmax denominators fall out of the PV
  matmul as one extra output row.
  scores (transposed): ST2[k, head, q] = KT_j.T @ QT per head into one
  2-bank PSUM tile, bf16 operands, exact causal widths (bf16 matmuls run
  1 cyc/row at any width). ONE exp per j covers both heads (halves ScalarE
  instruction-init overhead); 128x128 bf16 tri-mask multiply on DVE for
  diagonal tiles.
  PV: ctxT[65, q] += V_aug_j.T @ PT_j in bf16 (row 64 = denom).
  normalize off the critical path: evacuate ctx rows to SBUF (DVE) + denom
  row (ScalarE) so the PSUM bank frees fast; reciprocal_approx_fast, gpsimd
  partition-broadcast, one DVE multiply into ctx_all.
  O-projection in f32r: out[s, D] = sum_ct ctxT_ct.T @ woT_ct.

Schedule (chunk-major): attention stretches are exp-paced on ScalarE
(1.67W + ~190ns per j-step vs PE's 1.67W), so matmul-only fill work is
woven BETWEEN the four pairs' attentions of each stretch: the next chunk's
Q/K projections + V group in stretches 0..2 (which are then PE-bound), and
ALL the O-projection tiles of chunks 0..2 in the last stretch where the exp
backlog peaks. All fill PSUM lives on the "pp" banks so fills never block
the attention's st/ctx banks. DMA issues avoid the Activation queue (each
would tie its sequencer ~630ns) except a handful of early init-critical
transfers; v_sb is double-buffered so the next rep's V projection does not
serialize on this rep's final attention chunk.
"""
import sys
for _p in ('/opt/trn_rl_repo', '/root/.axon_site/_ro/trn_rl_repo'):
    if _p not in sys.path:
        sys.path.insert(0, _p)

import numpy as np

B, S, D, H = 4, 2048, 1024, 16
DH = 64
N_CORES = 8
HL = H // 2           # local heads per core
DL = HL * DH          # local ctx dims per core
SW = 32.0             # fp8 weight pre-scale for Q/K projections
QK_FP8 = True         # False: bf16 Q/K projections (safe fallback)


def build_nc(s=S, d=D, hl=HL, n_cores=N_CORES, reps=1):
    """Build the per-core Bass program (shapes parameterizable for sim tests)."""
    import concourse.bacc as bacc
    import concourse.mybir as mybir
    import concourse.tile as tile

    DT = mybir.dt
    F32 = DT.float32
    F32R = DT.float32r
    BF16 = DT.bfloat16
    F8 = DT.float8e4
    AFT = mybir.ActivationFunctionType
    PM = mybir.MatmulPerfMode

    dl = hl * DH
    n_kt = s // 128       # k/s tiles
    n_ch = s // 512       # 512-wide q chunks
    n_dt = d // 128       # d_model tiles
    n_oc = d // 512       # output d chunks
    pairs = hl // 2
    exp_scale = 0.125 / (SW * SW) if QK_FP8 else 0.125
    WDT = F8 if QK_FP8 else BF16

    nc = bacc.Bacc("TRN2", target_bir_lowering=False, debug=False,
                   num_devices=n_cores)
    # host-prepacked layouts (contiguous DMA blocks)
    xb = nc.declare_dram_parameter("xb", [n_dt, 128, s], BF16, isOutput=False)
    if QK_FP8:
        x8 = nc.declare_dram_parameter("x8", [n_dt, 128, s], F8, isOutput=False)
    wqP = nc.declare_dram_parameter("wqP", [128, pairs, n_dt, 128], WDT,
                                    isOutput=False)
    wkP = nc.declare_dram_parameter("wkP", [128, pairs, n_dt, 128], WDT,
                                    isOutput=False)
    wvP = nc.declare_dram_parameter("wvP", [n_dt, 128, dl], BF16, isOutput=False)
    woT = nc.declare_dram_parameter("woT", [dl, d], F32R, isOutput=False)
    tri = nc.declare_dram_parameter("tri", [128, 128], BF16, isOutput=False)
    out = nc.declare_dram_parameter("out", [s, d], F32, isOutput=True)

    with tile.TileContext(nc) as tc:
        with tc.tile_pool(name="persist", bufs=1) as pp, \
             tc.tile_pool(name="stream", bufs=1) as sp, \
             tc.tile_pool(name="psum", bufs=1, space="PSUM") as ps:

            # ---- resident tensors ----
            # v_sb: DH+2 bf16 cols per head so the ones column sits 4B-aligned
            # (cols DH:DH+2 memset as one packed f32; PV reads 0:DH+1)
            xb_sb = pp.tile([128, n_dt, s], BF16, name="xb_sb")
            if QK_FP8:
                x8_sb = pp.tile([128, n_dt, s], F8, name="x8_sb")
            else:
                x8_sb = xb_sb
            # per-chunk ctx tiles: exact dependencies, so O-projection of
            # finished chunks never waits on later chunks' normalize writes
            ctxs = [pp.tile([128, pairs, 512], F32R, name=f"ctx_{c}")
                    for c in range(n_ch)]
            qk_all = pp.tile([128, 2, pairs, s], BF16, name="qk_all")
            tri2 = pp.tile([128, 2, 128], BF16, name="tri2")

            nc.gpsimd.dma_start(out=tri2[:, 0, :], in_=tri[:, :])
            nc.gpsimd.dma_start(out=tri2[:, 1, :], in_=tri[:, :])

            for _rep in range(reps):
              R = f"{_rep}_" if reps > 1 else ""
              wq_sb = sp.tile([128, pairs, n_dt, 128], WDT, name=f"{R}wq_sb",
                              tag="wq")
              wk_sb = sp.tile([128, pairs, n_dt, 128], WDT, name=f"{R}wk_sb",
                              tag="wk")
              wv_sb = sp.tile([128, n_dt, dl], BF16, name=f"{R}wv_sb", tag="wv")
              wo_sb = sp.tile([128, pairs, d], F32R, name=f"{R}wo_sb", tag="wo")
              # double-buffered so rep r+1's V projection does not serialize
              # on rep r's final attention chunk (which reads all V tiles)
              v_sb = sp.tile([128, n_kt, hl, DH + 2], BF16, name=f"{R}v_sb",
                             tag="v", bufs=2)

              # ---- upfront DMA ----
              # The Activation queue must stay clear for the exp stream (each
              # DMA issue ties its sequencer ~630ns), but the first exp isn't
              # needed until ~15us in: let scalar take a handful of the
              # init-critical issues, then alternate sync (HWDGE) / gpsimd
              # (SWDGE).
              QQ = (nc.sync, nc.gpsimd)
              qi = [0]

              # preload the Exp activation table before scalar's DMA issues
              warm = sp.tile([1, 2], F32, name=f"{R}warm", tag="warm", bufs=1)
              nc.scalar.activation(out=warm, in_=tri2[0:1, 0, 0:2],
                                   func=AFT.Exp, scale=1.0)

              def dma(dst, src):
                  if qi[0] % 3 == 1 and qi[0] < 18:
                      nc.scalar.dma_start(out=dst, in_=src)
                  else:
                      QQ[(qi[0] % 3) > 0].dma_start(out=dst, in_=src)
                  qi[0] += 1

              # v_sb ones-columns first (Pool op; V evacs wait on it)
              # two bf16 ones packed as one f32 (bits 0x3F803F80)
              ones2 = float(np.frombuffer(np.uint32(0x3F803F80).tobytes(),
                                          dtype=np.float32)[0])
              nc.gpsimd.memset(v_sb[:, :, :, DH:DH + 2].bitcast(F32), ones2)

              # proj-chunk-0 gate first at fine granularity (pair-0 weights +
              # first x8 strips land within ~2.5us), then V-group-0 gate,
              # then the rest coarsened to one transfer per (tile, stream)
              for p_ in range(pairs):
                  dma(wq_sb[:, p_, :, :], wqP[:, p_, :, :])
                  if QK_FP8 and p_ * 2 < n_dt:
                      dma(x8_sb[:, 2 * p_, 0:512], x8[2 * p_, :, 0:512])
                      dma(x8_sb[:, 2 * p_ + 1, 0:512], x8[2 * p_ + 1, :, 0:512])
              for p_ in range(pairs):
                  dma(wk_sb[:, p_, :, :], wkP[:, p_, :, :])
              if QK_FP8:
                  for t in range(2 * pairs, n_dt):
                      dma(x8_sb[:, t, 0:512], x8[t, :, 0:512])
              for t in range(n_dt):
                  dma(wv_sb[:, t, :], wvP[t])
                  dma(xb_sb[:, t, 0:512], xb[t, :, 0:512])
              if n_ch > 1:
                  for t in range(n_dt):
                      dma(xb_sb[:, t, 512:s], xb[t, :, 512:s])
                      if QK_FP8:
                          dma(x8_sb[:, t, 512:s], x8[t, :, 512:s])
              dma(wo_sb,
                  woT.rearrange("(c r) d -> r c d", r=128))

              # ---- building blocks ----
              def proj(g, p, c4):
                  """Q (g=0) / K (g=1) projection of pair p, chunk c4."""
                  w_sb = wq_sb if g == 0 else wk_sb
                  psx = ps.tile([128, 512], F32, name=f"{R}ps{g}_{p}_{c4}",
                                tag="pp", bufs=2)
                  cs = slice(512 * c4, 512 * (c4 + 1))
                  if QK_FP8:
                      for t2 in range(n_dt // 2):
                          nc.tensor.matmul(psx[:, :],
                                           w_sb[:, p, 2 * t2:2 * t2 + 2, :],
                                           x8_sb[:, 2 * t2:2 * t2 + 2, cs],
                                           start=(t2 == 0),
                                           stop=(t2 == n_dt // 2 - 1),
                                           perf_mode=PM.DoubleRow)
                  else:
                      for t in range(n_dt):
                          nc.tensor.matmul(psx[:, :], w_sb[:, p, t, :],
                                           xb_sb[:, t, cs],
                                           start=(t == 0), stop=(t == n_dt - 1))
                  nc.vector.tensor_copy(out=qk_all[:, g, p, cs], in_=psx)

              def v_proj(kt):
                  pv = ps.tile([128, dl], F32, name=f"{R}pv_{kt}",
                               tag="pp", bufs=2)
                  for t in range(n_dt):
                      nc.tensor.matmul(pv[:, :],
                                       xb_sb[:, t, 128 * kt:128 * (kt + 1)],
                                       wv_sb[:, t, :],
                                       start=(t == 0), stop=(t == n_dt - 1))
                  nc.vector.tensor_copy(
                      out=v_sb[:, kt, :, 0:DH],
                      in_=pv.rearrange("p (h e) -> p h e", e=DH))

              def attn(p, c4):
                  q0 = 512 * c4
                  qt = qk_all[:, 0, p, :]
                  kt_sb = qk_all[:, 1, p, :]
                  ctxA = ps.tile([DH + 1, 512], F32, name=f"{R}cA_{p}_{c4}",
                                 tag="ctxA", bufs=1)
                  ctxB = ps.tile([DH + 1, 512], F32, name=f"{R}cB_{p}_{c4}",
                                 tag="ctxB", bufs=1)
                  nj = 4 * c4 + 4
                  pending = None
                  for j in range(nj):
                      m = j - 4 * c4
                      n0 = 128 * m if m >= 0 else 0
                      st2 = ps.tile([128, 2, 512], F32, name=f"{R}s_{p}_{c4}_{j}",
                                    tag="st", bufs=2)
                      ks = slice(128 * j, 128 * (j + 1))
                      qs = slice(q0 + n0, q0 + 512)
                      nc.tensor.matmul(st2[:, 0, n0:512], kt_sb[0:64, ks],
                                       qt[0:64, qs], start=True, stop=True)
                      nc.tensor.matmul(st2[:, 1, n0:512], kt_sb[64:128, ks],
                                       qt[64:128, qs], start=True, stop=True)
                      pt2 = sp.tile([128, 2, 512], BF16, name=f"{R}p_{p}_{c4}_{j}",
                                    tag="pt", bufs=3)
                      nc.scalar.activation(out=pt2[:, :, n0:512],
                                           in_=st2[:, :, n0:512],
                                           func=AFT.Exp, scale=exp_scale)
                      if m >= 0:
                          nc.vector.tensor_mul(pt2[:, :, n0:n0 + 128],
                                               pt2[:, :, n0:n0 + 128], tri2)
                      if pending is not None:
                          _emit_pv(nc, v_sb, ctxA, ctxB, p, pending, nj)
                      pending = (j, pt2, n0)
                  _emit_pv(nc, v_sb, ctxA, ctxB, p, pending, nj)

                  # evacuate ctx rows to SBUF scratch (frees the ctx bank
                  # after two short copies); normalize off the critical path:
                  # fast reciprocal of the denom row, gpsimd partition-
                  # broadcast, one multiply into the chunk's ctx tile.
                  # Steps are emitted for BOTH heads together so the two
                  # chains pipeline across DVE/Act/Pool. Partition offsets
                  # per op follow baseline-proven patterns (out==ins offset
                  # for TensorTensor; cross-offset only on TensorCopy).
                  cn = sp.tile([128, 512], F32, name=f"{R}cn_{p}_{c4}",
                               tag="cn", bufs=2)
                  dns, rbs = [], []
                  for head, cpsum in ((0, ctxA), (1, ctxB)):
                      hs = slice(64 * head, 64 * head + 64)
                      nc.vector.tensor_copy(out=cn[hs, :], in_=cpsum[0:DH, :])
                      dn1 = sp.tile([1, 512], F32, name=f"{R}dn_{p}_{c4}_{head}",
                                    tag="dn", bufs=2)
                      nc.vector.tensor_copy(out=dn1, in_=cpsum[DH:DH + 1, :])
                      dns.append(dn1)
                  for head in (0, 1):
                      rb1 = sp.tile([1, 512], F32, name=f"{R}rc_{p}_{c4}_{head}",
                                    tag="rc", bufs=2)
                      nc.vector.reciprocal_approx_fast(out=rb1, in_=dns[head])
                      rbs.append(rb1)
                  rbb = []
                  for head in (0, 1):
                      rb = sp.tile([128, 512], F32, name=f"{R}rb_{p}_{c4}_{head}",
                                   tag="rb", bufs=2)
                      nc.gpsimd.partition_broadcast(rb, rbs[head])
                      rbb.append(rb)
                  for head in (0, 1):
                      hs = slice(64 * head, 64 * head + 64)
                      nc.vector.tensor_mul(ctxs[c4][hs, p, :],
                                           cn[hs, :], rbb[head][hs, :])

              def o_proj(st_i):
                  ot = sp.tile([128, d], F32, name=f"{R}ot_{st_i}",
                               tag="ot", bufs=2)
                  for oc in range(n_oc):
                      pso = ps.tile([128, 512], F32, name=f"{R}po_{st_i}_{oc}",
                                    tag="pp", bufs=2)
                      sl = slice(128 * (st_i % 4), 128 * (st_i % 4) + 128)
                      for ct in range(pairs):
                          nc.tensor.matmul(
                              pso[:, :], ctxs[st_i // 4][:, ct, sl],
                              wo_sb[:, ct, 512 * oc:512 * (oc + 1)],
                              start=(ct == 0), stop=(ct == pairs - 1))
                      nc.vector.tensor_copy(
                          out=ot[:, 512 * oc:512 * (oc + 1)], in_=pso)
                      # flush each half as soon as it lands: halves the
                      # final drain tail
                      QQ[(2 * st_i + oc) % 2].dma_start(
                          out=out[128 * st_i:128 * (st_i + 1),
                                  512 * oc:512 * (oc + 1)],
                          in_=ot[:, 512 * oc:512 * (oc + 1)])

              # ---- chunk-major schedule ----
              # init: chunk-0 projections (smallest DMA gate), then V group 0
              for p in range(pairs):
                  proj(0, p, 0)
                  proj(1, p, 0)
              for kt in range(4):
                  v_proj(kt)

              # Attention stretches are Act-paced (exp: 1.67W + ~190ns per
              # j-step vs PE's 1.67W): PE idles at each pair's tail waiting
              # for the exp queue. Fill those bubbles with matmul-only work
              # (next chunk's projections + V group, previous chunk's O
              # tiles) spread BETWEEN the pair attentions of each stretch.
              # Stretches 0..n_ch-2 are PE-bound with just the next chunk's
              # projections + V group as fill; ALL O-projection fill goes to
              # the last stretch, where the Act (exp) backlog peaks.
              for c4 in range(n_ch):
                  fills = []
                  if c4 < n_ch - 1:
                      fills += [lambda g=g, p=p, c=c4 + 1: proj(g, p, c)
                                for p in range(pairs) for g in (0, 1)]
                      fills += [lambda kt=kt: v_proj(kt)
                                for kt in range(4 * (c4 + 1), 4 * (c4 + 2))]
                  else:
                      fills += [lambda st=st: o_proj(st)
                                for st in range(0, 4 * c4)]
                  for p in range(pairs):
                      attn(p, c4)
                      lo = len(fills) * p // pairs
                      hi = len(fills) * (p + 1) // pairs
                      for f in fills[lo:hi]:
                          f()
                  # last chunk: final O tiles come after the whole stretch
                  if c4 == n_ch - 1:
                      for st in range(4 * c4, 4 * c4 + 4):
                          o_proj(st)

    nc.compile()
    return nc


def _emit_pv(nc, v_sb, ctxA, ctxB, p, pending, nj):
    j, pt2, n0 = pending
    start = (j == 0)
    stop = (j == nj - 1)
    nc.tensor.matmul(ctxA[:, n0:512], v_sb[:, j, 2 * p, 0:DH + 1],
                     pt2[:, 0, n0:512], start=start, stop=stop)
    nc.tensor.matmul(ctxB[:, n0:512], v_sb[:, j, 2 * p + 1, 0:DH + 1],
                     pt2[:, 1, n0:512], start=start, stop=stop)


def make_tri():
    import ml_dtypes
    k = np.arange(128)[:, None]
    q = np.arange(128)[None, :]
    return (k <= q).astype(ml_dtypes.bfloat16)


def shard_inputs(in_features, q_weight, k_weight, v_weight, o_weight):
    """-> list of 8 per-core input dicts (host-side repack + quantize)."""
    import ml_dtypes
    BF = ml_dtypes.bfloat16
    F8 = ml_dtypes.float8_e4m3
    n_dt = D // 128
    pairs = HL // 2
    tri = make_tri()
    maps = []
    for c in range(N_CORES):
        b, g = divmod(c, 2)
        hs = slice(DL * g, DL * (g + 1))   # local head dims in the full D
        xT = in_features[b].T              # [D, S]
        xTt = np.ascontiguousarray(xT.reshape(n_dt, 128, S))
        wq = q_weight[hs, :]               # [DL, D]
        wk = k_weight[hs, :]
        # wP[r, p, t, m] = w[128p+m, 128t+r]
        wqP = np.ascontiguousarray(
            wq.reshape(pairs, 128, n_dt, 128).transpose(3, 0, 2, 1))
        wkP = np.ascontiguousarray(
            wk.reshape(pairs, 128, n_dt, 128).transpose(3, 0, 2, 1))
        m = {
            "xb": xTt.astype(BF),
            "wvP": np.ascontiguousarray(
                v_weight[hs, :].T.reshape(n_dt, 128, DL)).astype(BF),
            "woT": np.ascontiguousarray(o_weight[:, hs].T),
            "tri": tri,
        }
        if QK_FP8:
            m["x8"] = xTt.astype(F8)
            m["wqP"] = (wqP * SW).astype(F8)
            m["wkP"] = (wkP * SW).astype(F8)
        else:
            m["wqP"] = wqP.astype(BF)
            m["wkP"] = wkP.astype(BF)
        maps.append(m)
    return maps


def gather_output(results):
    """results: list of 8 dicts with 'out' [S, D] partials -> [B, S, D]."""
    return np.stack([results[2 * b]["out"] + results[2 * b + 1]["out"]
                     for b in range(B)])


_nc_cache = {}


def kernel(in_features, q_weight, k_weight, v_weight, o_weight):
    from concourse.bass_utils import run_bass_kernel_spmd
    if "nc" not in _nc_cache:
        _nc_cache["nc"] = build_nc()
    nc = _nc_cache["nc"]
    in_maps = shard_inputs(np.asarray(in_features, dtype=np.float32),
                           np.asarray(q_weight, dtype=np.float32),
                           np.asarray(k_weight, dtype=np.float32),
                           np.asarray(v_weight, dtype=np.float32),
                           np.asarray(o_weight, dtype=np.float32))
    res = run_bass_kernel_spmd(nc, in_maps, core_ids=list(range(N_CORES)))
    return gather_output(res.results)
